# revision 1
# baseline (speedup 1.0000x reference)
"""Multi-head attention (SuperGlue-style, conv1x1 projections) on 8 Trainium2
NeuronCores.

Sharding: pure data-parallel over batch (B=8 -> 1 batch element per core),
zero collectives. Weights replicated.

Per-core math (one batch element, x* = [D=256, N=2048], H=4 heads, dh=64):
  q = 0.125 * (Wq x + bq)   (score scale folded into q projection)
  k = Wk x + bk
  vT = x^T Wv^T             (v computed transposed: [n, dm] layout)
  per head h:
    S^T[m, n] = k_h[:, m]^T q_h[:, n]        (PE, K=64)
    E = exp(S^T)                              (ScalarE, no max subtraction:
                                               scores ~ N(0,1) so fp32-safe)
    num[d, n]  = sum_m v_aug[m, 65]^T E[m,n]  (PE, K=128; col 64 of v_aug is
                                               ones -> row 64 = softmax denom)
    x_h = num[0:64] * (1/num[64])             (DVE; recip broadcast down
                                               partitions via a K=1 PE
                                               outer-product into PSUM)
  out = Wm' x' + bm_eff     (per-head K=64 accumulation; bv folded into
                             bm_eff = bm + Wm bv since softmax rows sum to 1)

Head channels are interleaved in d_model (dm = i*H + h); all weight
permutations that make heads contiguous are applied on the host for free.

Performance notes (HW-measured on trn2 via axon):
- All matmul operands are float32r (TF32-like): 1 cycle/row on the PE vs 4
  for plain fp32. Rel err vs fp32 reference: 6.2e-4 (fp32 path was 3.7e-6
  but 4x slower on the PE). BIR requires every producer of a f32r-consumed
  tile to write f32r (hence f32r DRAM decls + f32r tile dtypes; memset
  can't write f32r, so the ones column comes from a broadcast copy).
- exp() on ScalarE is the floor: 16.8M elements/core at 1 elem/lane/cycle
  @1.2GHz + 352 cycles/instr overhead -> widest possible activations
  ([128,1024] from 2-bank PSUM tiles) matter.
- Softmax numerator accumulates in PSUM only per 4-m-chunk group, drained
  to an SBUF accumulator on DVE; this frees PSUM for a 3-deep S^T pipeline
  (psS bufs=3 x 2 banks + psN 1 x 2 banks = 8 banks exactly), which was
  worth 2.4x on HW (395us -> 166us steady-state per-core). Replacing the
  reciprocal DRAM-bounce broadcast with a K=1 PE outer product
  (ones[1,64]^T @ recip[1,512] -> PSUM) removed the last DMA latency from
  head boundaries; best quiet-window measurement ~122us/core.
- DMAs are spread across SP/ACT/gpsimd queues; descriptor generation on
  one queue sequencer (~25ns/partition-row) otherwise serializes.
"""

import numpy as np
from contextlib import ExitStack

import concourse.bass as bass
import concourse.tile as tile
from concourse import bacc, mybir
from concourse.bass_utils import run_bass_kernel_spmd

B, D, N, H = 8, 256, 2048, 4
DH = D // H            # 64 per-head channels
PC = 128               # partition chunk
KC = D // PC           # 2 contraction chunks for convs
NT = 512               # free-dim tile (fp32 matmul moving max)
NNT = N // NT          # 4 n-tiles
MC = N // PC           # 16 m-chunks (key/seq chunks on partitions)
VA_W = DH + 1          # 65: per-head v^T columns + ones column
F32 = mybir.dt.float32
F32R = mybir.dt.float32r


def mm(ap):
    """Matmul operands live in float32r tiles (full PE rate; fp32 is 4
    cycles/row). Producers must write f32r-rounded values (BIR rule)."""
    return ap


def emit(ctx: ExitStack, tc: tile.TileContext, io: dict):
    nc = tc.nc
    xq, xk, xv = io["xq"], io["xk"], io["xv"]
    wqT, wkT, wvT, wmT = io["wqT"], io["wkT"], io["wvT"], io["wmT"]
    bq, bk, bm = io["bq"], io["bk"], io["bm"]
    rs = io["rs"]
    out = io["out"]

    consts = ctx.enter_context(tc.tile_pool(name="consts", bufs=1))
    in_pool = ctx.enter_context(tc.tile_pool(name="in_pool", bufs=4))
    qk_pool = ctx.enter_context(tc.tile_pool(name="qk_pool", bufs=2))
    va_pool = ctx.enter_context(tc.tile_pool(name="va_pool", bufs=MC))
    e_pool = ctx.enter_context(tc.tile_pool(name="e_pool", bufs=6))
    x_pool = ctx.enter_context(tc.tile_pool(name="x_pool", bufs=4))
    sm_pool = ctx.enter_context(tc.tile_pool(name="sm_pool", bufs=4))
    bc_pool = ctx.enter_context(tc.tile_pool(name="bc_pool", bufs=4))
    out_pool = ctx.enter_context(tc.tile_pool(name="out_pool", bufs=2))
    psS = ctx.enter_context(tc.tile_pool(name="psS", bufs=3, space="PSUM"))
    psN = ctx.enter_context(tc.tile_pool(name="psN", bufs=1, space="PSUM"))
    acc_pool = ctx.enter_context(tc.tile_pool(name="acc_pool", bufs=4))

    # --- weights / biases to SBUF ---
    w_q = [consts.tile([PC, D], F32R, tag=f"wq{kc}", name="wq") for kc in range(KC)]
    w_k = [consts.tile([PC, D], F32R, tag=f"wk{kc}", name="wk") for kc in range(KC)]
    w_v = [consts.tile([PC, D], F32R, tag=f"wv{kc}", name="wv") for kc in range(KC)]
    for kc in range(KC):
        nc.gpsimd.dma_start(w_q[kc][:], wqT[kc * PC:(kc + 1) * PC, :])
        nc.gpsimd.dma_start(w_k[kc][:], wkT[kc * PC:(kc + 1) * PC, :])
        nc.gpsimd.dma_start(w_v[kc][:], wvT[kc * PC:(kc + 1) * PC, :])
    w_m = [consts.tile([DH, D], F32R, tag=f"wm{h}", name="wm") for h in range(H)]
    for h in range(H):
        nc.gpsimd.dma_start(w_m[h][:], wmT[h * DH:(h + 1) * DH, :])
    ones_sb = consts.tile([PC, 1], F32R, tag="ones", name="ones")
    nc.gpsimd.dma_start(ones_sb[:], io["onec"].partition_broadcast(PC))
    ones_row = consts.tile([1, DH], F32R, tag="onesr", name="onesr")
    nc.vector.tensor_copy(ones_row[:], ones_sb[0:1, 0:1].broadcast_to([1, DH]))
    b_q = [consts.tile([PC, 1], F32, tag=f"bq{oc}", name="bq") for oc in range(KC)]
    b_k = [consts.tile([PC, 1], F32, tag=f"bk{oc}", name="bk") for oc in range(KC)]
    b_m = [consts.tile([PC, 1], F32, tag=f"bm{oc}", name="bm") for oc in range(KC)]
    for oc in range(KC):
        nc.gpsimd.dma_start(b_q[oc][:], bq[oc * PC:(oc + 1) * PC, :])
        nc.gpsimd.dma_start(b_k[oc][:], bk[oc * PC:(oc + 1) * PC, :])
        nc.gpsimd.dma_start(b_m[oc][:], bm[oc * PC:(oc + 1) * PC, :])

    # --- load activations ---
    x_in = {}
    for name, dram, eng in (
        ("xq", xq, nc.sync), ("xk", xk, nc.scalar), ("xv", xv, nc.gpsimd)
    ):
        x_in[name] = [in_pool.tile([PC, N], F32R, tag="xin", name="xin") for _ in range(KC)]
        for kc in range(KC):
            eng.dma_start(x_in[name][kc][:], dram[kc * PC:(kc + 1) * PC, :])

    # --- Q / K projections: out[o', n] = sum_i W^T[i, o'] x[i, n] (+ bias) ---
    q_sb = [qk_pool.tile([PC, N], F32R, tag="qsb", name="qsb") for _ in range(KC)]
    k_sb = [qk_pool.tile([PC, N], F32R, tag="ksb", name="ksb") for _ in range(KC)]
    for w_sb, b_sb, x_sb_in, dst in (
        (w_q, b_q, x_in["xq"], q_sb),
        (w_k, b_k, x_in["xk"], k_sb),
    ):
        for oc in range(KC):
            for nt in range(NNT):
                ps = psS.tile([PC, NT], F32, tag="sps", name="cps")
                for kc in range(KC):
                    nc.tensor.matmul(
                        ps[:],
                        lhsT=mm(w_sb[kc][:, oc * PC:(oc + 1) * PC]),
                        rhs=mm(x_sb_in[kc][:, nt * NT:(nt + 1) * NT]),
                        start=(kc == 0),
                        stop=(kc == KC - 1),
                    )
                nc.vector.tensor_scalar_add(
                    dst[oc][:, nt * NT:(nt + 1) * NT], ps[:], b_sb[oc][:]
                )

    # --- V^T projection + ones column: va[mc] = [128(m), H*65] ---
    va = [va_pool.tile([PC, H * VA_W], F32R, tag="va", name="va") for _ in range(MC)]
    for mc in range(MC):
        ps = psS.tile([PC, D], F32, tag="sps", name="cps")
        for kc in range(KC):
            nc.tensor.matmul(
                ps[:],
                lhsT=mm(x_in["xv"][kc][:, mc * PC:(mc + 1) * PC]),
                rhs=mm(w_v[kc][:]),
                start=(kc == 0),
                stop=(kc == KC - 1),
            )
        ones_cols = va[mc][:].rearrange("p (h w) -> p h w", h=H)[:, :, DH]
        nc.vector.tensor_copy(ones_cols, ones_sb[:].broadcast_to([PC, H]))
        for h in range(H):
            nc.vector.tensor_copy(
                va[mc][:, h * VA_W:h * VA_W + DH],
                ps[:, h * DH:(h + 1) * DH],
            )

    # --- attention per head, n-tiles processed in pairs (1024-wide exp) ---
    x_att = [x_pool.tile([DH, N], F32R, tag="xatt", name="xatt") for _ in range(H)]
    for h in range(H):
        tix = h // 2          # which q/k tile holds this head
        hb = (h % 2) * DH     # partition base of this head inside the tile
        for half in range(2):
            nts = (2 * half, 2 * half + 1)
            GRP = 4
            acc = [acc_pool.tile([VA_W, NT], F32, tag="acc", name="acc")
                   for _ in nts]
            for g in range(MC // GRP):
                nps = psN.tile([VA_W, 2 * NT], F32, tag="nps", name="nps")
                for mc in range(g * GRP, (g + 1) * GRP):
                    sps = psS.tile([PC, 2 * NT], F32, tag="sps", name="sps")
                    for j, nt in enumerate(nts):
                        nc.tensor.matmul(
                            sps[:, j * NT:(j + 1) * NT],
                            lhsT=mm(k_sb[tix][hb:hb + DH, mc * PC:(mc + 1) * PC]),
                            rhs=mm(q_sb[tix][hb:hb + DH, nt * NT:(nt + 1) * NT]),
                            start=True,
                            stop=True,
                        )
                    e_t = e_pool.tile([PC, 2 * NT], F32R, tag="et", name="et")
                    nc.scalar.activation(e_t[:], sps[:],
                                         mybir.ActivationFunctionType.Exp)
                    for j, nt in enumerate(nts):
                        nc.tensor.matmul(
                            nps[:, j * NT:(j + 1) * NT],
                            lhsT=mm(va[mc][:, h * VA_W:(h + 1) * VA_W]),
                            rhs=mm(e_t[:, j * NT:(j + 1) * NT]),
                            start=(mc % GRP == 0),
                            stop=(mc % GRP == GRP - 1),
                        )
                for j in range(2):
                    seg = nps[:, j * NT:(j + 1) * NT]
                    if g == 0:
                        nc.vector.tensor_copy(acc[j][:], seg)
                    else:
                        nc.vector.tensor_add(acc[j][:], acc[j][:], seg)
            ps_b = psN.tile([VA_W, 2 * NT], F32, tag="nps", name="bps")
            for j, nt in enumerate(nts):
                r = sm_pool.tile([1, NT], F32R, tag="recip", name="recip")
                with nc.allow_low_precision(reason="f32r is fp32-width"):
                    nc.vector.reciprocal(r[:], acc[j][DH:DH + 1, :])
                # broadcast recip down 64 partitions: ones[1,64]^T @ r[1,512]
                nc.tensor.matmul(
                    ps_b[0:DH, j * NT:(j + 1) * NT],
                    lhsT=mm(ones_row[:]),
                    rhs=mm(r[:]),
                    start=True,
                    stop=True,
                )
                nc.vector.tensor_mul(
                    x_att[h][:, nt * NT:(nt + 1) * NT],
                    acc[j][0:DH, :],
                    ps_b[0:DH, j * NT:(j + 1) * NT],
                )

    # --- merge projection: out[o, n] = sum_h Wm'^T[h] x_h (+ bm_eff) ---
    for oc in range(KC):
        o_t = out_pool.tile([PC, N], F32, tag="ot", name="ot")
        for nt in range(NNT):
            ps = psS.tile([PC, NT], F32, tag="sps", name="cps")
            for h in range(H):
                nc.tensor.matmul(
                    ps[:],
                    lhsT=mm(w_m[h][:, oc * PC:(oc + 1) * PC]),
                    rhs=mm(x_att[h][:, nt * NT:(nt + 1) * NT]),
                    start=(h == 0),
                    stop=(h == H - 1),
                )
            nc.vector.tensor_scalar_add(
                o_t[:, nt * NT:(nt + 1) * NT], ps[:], b_m[oc][:]
            )
        nc.sync.dma_start(out[oc * PC:(oc + 1) * PC, :], o_t[:])


def build_nc(reps=1):
    nc = bacc.Bacc("TRN2", target_bir_lowering=False, debug=False, num_devices=B)
    io = {
        "xq": nc.dram_tensor("xq", [D, N], F32R, kind="ExternalInput").ap(),
        "xk": nc.dram_tensor("xk", [D, N], F32R, kind="ExternalInput").ap(),
        "xv": nc.dram_tensor("xv", [D, N], F32R, kind="ExternalInput").ap(),
        "wqT": nc.dram_tensor("wqT", [D, D], F32R, kind="ExternalInput").ap(),
        "wkT": nc.dram_tensor("wkT", [D, D], F32R, kind="ExternalInput").ap(),
        "wvT": nc.dram_tensor("wvT", [D, D], F32R, kind="ExternalInput").ap(),
        "wmT": nc.dram_tensor("wmT", [D, D], F32R, kind="ExternalInput").ap(),
        "bq": nc.dram_tensor("bq", [D, 1], F32, kind="ExternalInput").ap(),
        "bk": nc.dram_tensor("bk", [D, 1], F32, kind="ExternalInput").ap(),
        "bm": nc.dram_tensor("bm", [D, 1], F32, kind="ExternalInput").ap(),
        "rs": nc.dram_tensor("rs", [H * NNT, NT], F32).ap(),
        "onec": nc.dram_tensor("onec", [1, 1], F32R, kind="ExternalInput").ap(),
        "out": nc.dram_tensor("out", [D, N], F32, kind="ExternalOutput").ap(),
    }
    with tile.TileContext(nc) as tc:
        if reps == 1:
            with ExitStack() as ctx:
                emit(ctx, tc, io)
        else:
            with tc.For_i(0, reps, 1):
                with ExitStack() as ctx:
                    emit(ctx, tc, io)
    nc.compile()
    return nc


def host_inputs(query, key, value, Wq, bq, Wk, bk, Wv, bv, Wm, bm):
    """Host-side prep: head-deinterleaving permutation + scale/bias folding.

    Returns (shared weight map, list of per-core input maps)."""
    f = np.float32
    t = np.arange(D)
    perm = (t % DH) * H + t // DH  # row t = head-major channel -> original dm

    Wq = np.asarray(Wq, f); Wk = np.asarray(Wk, f); Wv = np.asarray(Wv, f)
    Wm = np.asarray(Wm, f)
    bq = np.asarray(bq, f); bk = np.asarray(bk, f); bv = np.asarray(bv, f)
    bm = np.asarray(bm, f)

    scale = f(1.0 / np.sqrt(DH))
    shared = {
        "onec": np.ones((1, 1), f),
        "wqT": np.ascontiguousarray(Wq.T[:, perm] * scale),
        "wkT": np.ascontiguousarray(Wk.T[:, perm]),
        "wvT": np.ascontiguousarray(Wv.T[:, perm]),
        "wmT": np.ascontiguousarray(Wm.T[perm, :]),
        "bq": np.ascontiguousarray((bq[perm] * scale).reshape(D, 1)),
        "bk": np.ascontiguousarray(bk[perm].reshape(D, 1)),
        "bm": np.ascontiguousarray((bm + Wm @ bv).reshape(D, 1)),
    }
    query = np.asarray(query, f); key = np.asarray(key, f)
    value = np.asarray(value, f)
    in_maps = []
    for b in range(B):
        m = dict(shared)
        m["xq"] = np.ascontiguousarray(query[b])
        m["xk"] = np.ascontiguousarray(key[b])
        m["xv"] = np.ascontiguousarray(value[b])
        in_maps.append(m)
    return shared, in_maps


_NC = None


def get_nc():
    global _NC
    if _NC is None:
        _NC = build_nc()
    return _NC


def kernel(query, key, value, Wq, bq, Wk, bk, Wv, bv, Wm, bm):
    nc = get_nc()
    _, in_maps = host_inputs(query, key, value, Wq, bq, Wk, bk, Wv, bv, Wm, bm)
    res = run_bass_kernel_spmd(nc, in_maps, core_ids=list(range(B)))
    return np.stack([res.results[b]["out"] for b in range(B)], axis=0)



# revision 14
# speedup vs baseline: 1.2992x; 1.2992x over previous
"""Multi-head attention (SuperGlue-style, conv1x1 projections) on 8 Trainium2
NeuronCores.

Sharding: pure data-parallel over batch (B=8 -> 1 batch element per core),
zero collectives. Weights replicated.

Per-core math (one batch element, x* = [D=256, N=2048], H=4 heads, dh=64):
  q = 0.125 * (Wq x + bq)   (score scale folded into q projection)
  k = Wk x + bk
  vT = x^T Wv^T             (v computed transposed: [n, dm] layout)
  per head h:
    S^T[m, n] = k_h[:, m]^T q_h[:, n]        (PE, K=64)
    E = exp(S^T)                              (ScalarE -> bf16; scores ~
                                               N(0,1) so fp32-safe, no max)
    num[d, n]  = sum_m v_aug[m, 65]^T E[m,n]  (PE, K=128; col 64 of v_aug is
                                               ones -> row 64 = softmax denom)
    x_h = num[0:64] * (1/num[64])             (DVE; recip broadcast down
                                               partitions via a K=1 PE
                                               outer-product into PSUM)
  out = Wm' x' + bm_eff     (head-PAIRED K=128 accumulation; bv folded into
                             bm_eff = bm + Wm bv since softmax rows sum to 1)

Head channels are interleaved in d_model (dm = i*H + h); all weight
permutations that make heads contiguous are applied on the host for free.

Precision (empirically validated vs fp32 reference, gate 2e-2):
- bf16 for DRAM->SBUF inputs, all weights, and E (exp output): these halve
  DMA + SBUF and cost ~2e-3 rel err each. q/k, va, x_att stay float32r
  (bf16 q/k alone costs ~1e-2). Measured combo: ~5e-3. PE rate is identical
  (1 cycle/row) for bf16 and f32r, so this trades only memory, not speed.
- fp8 DoubleRow (the only 2x PE mode) measured 2.5e-2..1.3e-1 on this
  metric for every placement -> unusable.

Schedule (the v2 rewrite; v1 modeled 201us/iter, v2 targets ~145us):
- Both PE (~131us: 311k cycles) and ACT (~133us: 128 exps of [128,1024] at
  1038ns) are near the 16.7M-scores/core roofline; everything else is
  subordinate to keeping ACT 100% busy and PE out of head-of-line stalls.
- Softmax numerator accumulates ACROSS ALL 16 m-chunks in PSUM (no DVE
  group drains -- v1 burned 91us of DVE on drains/copies, v2 ~45us).
- Numerator matmuls are deferred one chunk (S(mc+1) issues before N(mc)) so
  the in-order PE queue never waits on exp(mc).
- The recip-broadcast + normalize of unit u is emitted inside unit u+1's
  chunk stream (after chunk 1) for the same reason.
- PSUM: sps 2x[128,1024] (4 banks) + nps 2x[65,512] (2) + bps 1x[64,1024]
  (2) = 8 banks exactly.
- All weights arrive in ONE packed DMA (v1: ~19 gpsimd DMAs at ~1us fixed
  cost each); inputs use one tag per tensor so pool rotation never
  serializes (v1 shared one tag across 6 tiles with bufs=4).
"""

import numpy as np
from contextlib import ExitStack

import ml_dtypes

import concourse.bass as bass
import concourse.tile as tile
from concourse import bacc, mybir
from concourse.bass_utils import run_bass_kernel_spmd

B, D, N, H = 8, 256, 2048, 4
DH = D // H            # 64 per-head channels
PC = 128               # partition chunk
KC = D // PC           # 2 contraction chunks for convs
NT = 512               # free-dim tile
NNT = N // NT          # 4 n-tiles
MC = N // PC           # 16 m-chunks (key/seq chunks on partitions)
VA_W = DH + 1          # 65: per-head v^T columns + ones column
F32 = mybir.dt.float32
F32R = mybir.dt.float32r
BF16 = mybir.dt.bfloat16
NPBF16 = mybir.dt.np(BF16)


def emit(ctx: ExitStack, tc: tile.TileContext, io: dict):
    nc = tc.nc
    xq_d, xk_d, xv_d = io["xq"], io["xk"], io["xv"]
    wpack, bpack = io["wpack"], io["bpack"]
    out = io["out"]
    Exp = mybir.ActivationFunctionType.Exp

    consts = ctx.enter_context(tc.tile_pool(name="consts", bufs=1))
    in_pool = ctx.enter_context(tc.tile_pool(name="in_pool", bufs=2))
    qk_pool = ctx.enter_context(tc.tile_pool(name="qk_pool", bufs=1))
    va_pool = ctx.enter_context(tc.tile_pool(name="va_pool", bufs=MC))
    e_pool = ctx.enter_context(tc.tile_pool(name="e_pool", bufs=6))
    x_pool = ctx.enter_context(tc.tile_pool(name="x_pool", bufs=1))
    sm_pool = ctx.enter_context(tc.tile_pool(name="sm_pool", bufs=2))
    out_pool = ctx.enter_context(tc.tile_pool(name="out_pool", bufs=2))
    # PSUM ledger (8 banks): sps 2x[128,1024]=4, cps 2x[128,512]=2,
    # nps 2x[128,512]=2. nps rows 0..64 hold the numerator accumulation +
    # denominator row; rows 64..127 are reused as the recip-broadcast target.
    psP = ctx.enter_context(tc.tile_pool(name="psP", bufs=2, space="PSUM"))

    # --- constants: one packed weight DMA + one bias/ones DMA ---
    wsb = consts.tile([PC, 8 * D], BF16, tag="wsb", name="wsb", bufs=2)
    nc.sync.dma_start(wsb[:, 0:4 * D], wpack[:, 0:4 * D])
    bsb = consts.tile([PC, 7], F32, tag="bsb", name="bsb", bufs=2)
    nc.sync.dma_start(bsb[:], bpack[:, :])
    nc.sync.dma_start(wsb[:, 4 * D:8 * D], wpack[:, 4 * D:8 * D])

    def wslot(s):  # wpack slot s -> [PC, D] view
        return wsb[:, s * D:(s + 1) * D]

    ones_r = consts.tile([1, DH], F32R, tag="onesr", name="onesr")
    nc.vector.tensor_copy(ones_r[:], bsb[0:1, 6:7].broadcast_to([1, DH]))
    ones_b = consts.tile([PC, H], BF16, tag="onesb", name="onesb")
    nc.vector.tensor_copy(ones_b[:], bsb[:, 6:7].broadcast_to([PC, H]))

    # --- load activations: one [PC, 2, N] tile per input (c-chunk in free) ---
    x_in = {}
    HN = N // 2
    for name, dram, eng in (
        ("xq", xq_d, nc.scalar), ("xk", xk_d, nc.sync), ("xv", xv_d, nc.gpsimd)
    ):
        t = in_pool.tile([PC, KC * N], BF16, tag=name, name=name)
        if name == "xv":
            for kc in range(KC):
                eng.dma_start(t[:, kc * N:(kc + 1) * N],
                              dram[kc * PC:(kc + 1) * PC, :])
        else:
            for ch in range(2):          # column half ch: head needs ch=0 only
                for kc in range(KC):
                    eng.dma_start(
                        t[:, kc * N + ch * HN:kc * N + (ch + 1) * HN],
                        dram[kc * PC:(kc + 1) * PC, ch * HN:(ch + 1) * HN],
                    )
        x_in[name] = t

    # --- Q / K projections: out[o', n] = sum_i W^T[i, o'] x[i, n] (+ bias) ---
    # oc=0 (heads 0,1) is emitted up front -- it gates the first scores.
    # oc=1 (heads 2,3) is drip-fed into unit 1's chunk stream (fillers).
    q_sb = [qk_pool.tile([PC, N], F32R, tag=f"qsb{oc}", name="qsb") for oc in range(KC)]
    k_sb = [qk_pool.tile([PC, N], F32R, tag=f"ksb{oc}", name="ksb") for oc in range(KC)]

    def proj_mm(w_base, xt, oc, nt, kc):
        def f(ps):
            nc.tensor.matmul(
                ps[:],
                lhsT=wslot(w_base + kc)[:, oc * PC:(oc + 1) * PC],
                rhs=xt[:, kc * N + nt * NT:kc * N + (nt + 1) * NT],
                start=(kc == 0),
                stop=(kc == KC - 1),
            )
        return f

    def proj_nt(w_base, b_base, xt, dst, oc, nt):
        """Emit one [PC, NT] projection column block (2 matmuls + bias)."""
        ps = psP.tile([PC, NT], F32, tag="cps", name="cps")
        for kc in range(KC):
            proj_mm(w_base, xt, oc, nt, kc)(ps)
        nc.vector.tensor_scalar_add(
            dst[oc][:, nt * NT:(nt + 1) * NT], ps[:],
            bsb[:, b_base + oc:b_base + oc + 1],
        )

    for nt in range(2):
        proj_nt(0, 0, x_in["xq"], q_sb, 0, nt)
        proj_nt(2, 2, x_in["xk"], k_sb, 0, nt)

    # --- V^T projection + ones column: va[mc] = [128(m), H*65] ---
    # Emitted one m-chunk per chunk-slot inside unit 0 (fills PE slack while
    # the exp pipeline warms; va[mc] is ready 2 chunks before N(mc) needs it).
    va = [va_pool.tile([PC, H * VA_W], BF16, tag="va", name="va") for _ in range(MC)]

    def v_chunk(mc):
        ps = psP.tile([PC, D], F32, tag="cps", name="vps")
        for kc in range(KC):
            nc.tensor.matmul(
                ps[:],
                lhsT=x_in["xv"][:, kc * N + mc * PC:kc * N + (mc + 1) * PC],
                rhs=wslot(4 + kc),
                start=(kc == 0),
                stop=(kc == KC - 1),
            )
        va_v = va[mc][:].rearrange("p (h w) -> p h w", h=H)
        nc.vector.tensor_copy(
            va_v[:, :, 0:DH], ps[:].rearrange("p (h w) -> p h w", h=H)
        )
        nc.vector.tensor_copy(va_v[:, :, DH], ones_b[:])

    # --- merge: out[o, n] = sum_pair Wm'^T[pair] x_att[pair] (+ bm_eff) ---
    o_t = [out_pool.tile([PC, N], F32, tag="ot", name="ot") for _ in range(KC)]

    def merge_nt(oc, nt):
        ps = psP.tile([PC, NT], F32, tag="cps", name="mps")
        for p in range(KC):
            nc.tensor.matmul(
                ps[:],
                lhsT=wslot(6 + p)[:, oc * PC:(oc + 1) * PC],
                rhs=x_att[p][:, nt * NT:(nt + 1) * NT],
                start=(p == 0),
                stop=(p == KC - 1),
            )
        nc.vector.tensor_scalar_add(
            o_t[oc][:, nt * NT:(nt + 1) * NT], ps[:], bsb[:, 4 + oc:5 + oc]
        )

    def out_dma(oc, half):
        nc.sync.dma_start(
            out[oc * PC:(oc + 1) * PC, half * 2 * NT:(half + 1) * 2 * NT],
            o_t[oc][:, half * 2 * NT:(half + 1) * 2 * NT],
        )

    def out_dma_nt(oc, nt):
        nc.sync.dma_start(
            out[oc * PC:(oc + 1) * PC, nt * NT:(nt + 1) * NT],
            o_t[oc][:, nt * NT:(nt + 1) * NT],
        )

    # --- attention: 8 units of (head, 1024-wide half), software-pipelined.
    # Units run half-major so all of n[0:1024] finishes after 4 units and the
    # first merge half overlaps units 5-7. Numerator matmuls are deferred two
    # chunks; the last two N-pairs + recip + normalize of unit u are carried
    # as `pending` work pulled one item per chunk inside unit u+1, so the PE
    # never head-of-line blocks the ACT exp stream at unit boundaries.
    x_att = [x_pool.tile([PC, N], BF16, tag=f"xatt{p}", name="xatt")
             for p in range(KC)]
    pending = []   # closures: one pulled per chunk slot

    def unit(h, half, fillers):
        tix = h // 2          # which q/k tile holds this head
        hb = (h % 2) * DH     # partition base of this head inside the tile
        n0 = half * 2 * NT
        nps = [psP.tile([VA_W, NT], F32, tag="nps", name="nps") for _ in range(2)]
        e_ts = [None] * MC

        def n_mm(pm):
            for j in range(2):
                nc.tensor.matmul(
                    nps[j][:],
                    lhsT=va[pm][:, h * VA_W:(h + 1) * VA_W],
                    rhs=e_ts[pm][:, j * NT:(j + 1) * NT],
                    start=(pm == 0),
                    stop=(pm == MC - 1),
                )

        for mc in range(MC):
            sps = psP.tile([PC, 2 * NT], F32, tag="sps", name="sps")
            for j in range(2):
                nc.tensor.matmul(
                    sps[:, j * NT:(j + 1) * NT],
                    lhsT=k_sb[tix][hb:hb + DH, mc * PC:(mc + 1) * PC],
                    rhs=q_sb[tix][hb:hb + DH, n0 + j * NT:n0 + (j + 1) * NT],
                    start=True,
                    stop=True,
                )
            e_t = e_pool.tile([PC, 2 * NT], BF16, tag="et", name="et")
            nc.scalar.activation(e_t[:], sps[:], Exp)
            e_ts[mc] = e_t
            if mc < len(fillers):
                for f in fillers[mc]:
                    f()
            if pending:
                pending.pop(0)()
            if mc >= 4:
                n_mm(mc - 4)

        r = sm_pool.tile([1, 2 * NT], F32R, tag="recip", name="recip")
        num_sb = sm_pool.tile([DH, 2 * NT], F32R, tag="numsb", name="numsb")

        def tail_na():
            n_mm(MC - 4)
            n_mm(MC - 3)

        def tail_nb():
            n_mm(MC - 2)
            n_mm(MC - 1)

        def tail_recips():
            # recip of the denominator row + numerator drain to SBUF (DVE
            # can read only ONE PSUM operand per op, and draining here
            # releases the nps banks before the next unit's accumulation).
            for j in range(2):
                with nc.allow_low_precision(reason="f32r is fp32-width"):
                    nc.vector.reciprocal(r[:, j * NT:(j + 1) * NT],
                                         nps[j][DH:DH + 1, :])
            for j in range(2):
                nc.vector.tensor_copy(num_sb[:, j * NT:(j + 1) * NT],
                                      nps[j][0:DH, :])

        def norm():
            # broadcast recip down 64 partitions: ones[1,64]^T @ r[1,512]
            # into a cps-tag PSUM tile; multiply against the SBUF-drained
            # numerator (SBUF x PSUM -- the only legal DVE pairing).
            for j in range(2):
                bps = psP.tile([DH, NT], F32, tag="cps", name="bps")
                nc.tensor.matmul(
                    bps[:], lhsT=ones_r[:], rhs=r[:, j * NT:(j + 1) * NT],
                    start=True, stop=True,
                )
                nc.vector.tensor_mul(
                    x_att[tix][hb:hb + DH, n0 + j * NT:n0 + (j + 1) * NT],
                    num_sb[:, j * NT:(j + 1) * NT],
                    bps[:],
                )
        pending.extend([tail_na, tail_nb, tail_recips, norm])

    # Deferred projection work, drip-fed as per-chunk fillers with deadlines
    # set by when scores/numerators first read each block:
    #   unit 0: V chunks (va[mc] due at chunk mc+4) + k-oc0 nt2/nt3 (due at
    #           chunks 8/12); unit 1: q-oc1 + k-oc1-nt0 (due unit 2) and
    #           q-oc0 nt2/3 (due unit 4); unit 2: k-oc1 nt1-3 (due chunks
    #           4/8/12); unit 5: first merge half + its output DMA.
    def proj_ab(w_base, b_base, xt, dst, oc, nt):
        box = []

        def f_a():
            ps = psP.tile([PC, NT], F32, tag="cps", name="cps")
            box.append(ps)
            proj_mm(w_base, xt, oc, nt, 0)(ps)

        def f_b():
            ps = box.pop()
            proj_mm(w_base, xt, oc, nt, 1)(ps)
            nc.vector.tensor_scalar_add(
                dst[oc][:, nt * NT:(nt + 1) * NT], ps[:],
                bsb[:, b_base + oc:b_base + oc + 1],
            )
        return f_a, f_b

    Q, KW = (0, 0, x_in["xq"], q_sb), (2, 2, x_in["xk"], k_sb)
    k2a, k2b = proj_ab(*KW, 0, 2)
    k3a, k3b = proj_ab(*KW, 0, 3)
    f_u0 = [[k2a], [k2b], [k3a], [k3b]]
    for mc in range(MC - 1):
        slot = mc + 1
        if slot < len(f_u0):
            f_u0[slot].insert(0, lambda mc=mc: v_chunk(mc))
        else:
            f_u0.append([lambda mc=mc: v_chunk(mc)])

    f_u1 = [[lambda: v_chunk(MC - 1)]]
    for blk in (proj_ab(*Q, 1, 0), proj_ab(*Q, 1, 1), proj_ab(*KW, 1, 0),
                proj_ab(*Q, 0, 2), proj_ab(*Q, 0, 3)):
        f_u1 += [[blk[0]], [blk[1]]]

    f_u2 = []
    for blk in ([proj_ab(*KW, 1, nt) for nt in range(1, NNT)]
                + [proj_ab(*Q, 1, nt) for nt in range(2, NNT)]):
        f_u2 += [[blk[0]], [blk[1]]]

    f_mrg0 = [[lambda oc=oc, nt=nt: merge_nt(oc, nt)]
              for oc in range(KC) for nt in range(2)]
    f_mrg0 += [[lambda: out_dma(0, 0)], [lambda: out_dma(1, 0)]]

    unit_fillers = {0: f_u0, 1: f_u1, 2: f_u2, 5: f_mrg0}
    for u, (half, h) in enumerate((hf, hh) for hf in range(2) for hh in range(H)):
        unit(h, half, unit_fillers.get(u, []))
    for p in pending:   # last unit's N-tail + normalize
        p()
    pending.clear()

    # --- merge half 1 + drain (nt2 columns are ready first) ---
    for nt in range(2, NNT):
        for oc in range(KC):
            merge_nt(oc, nt)
        for oc in range(KC):
            out_dma_nt(oc, nt)


def build_nc(reps=1):
    nc = bacc.Bacc("TRN2", target_bir_lowering=False, debug=False, num_devices=B)
    io = {
        "xq": nc.dram_tensor("xq", [D, N], BF16, kind="ExternalInput").ap(),
        "xk": nc.dram_tensor("xk", [D, N], BF16, kind="ExternalInput").ap(),
        "xv": nc.dram_tensor("xv", [D, N], BF16, kind="ExternalInput").ap(),
        "wpack": nc.dram_tensor("wpack", [PC, 8 * D], BF16, kind="ExternalInput").ap(),
        "bpack": nc.dram_tensor("bpack", [PC, 7], F32, kind="ExternalInput").ap(),
        "out": nc.dram_tensor("out", [D, N], F32, kind="ExternalOutput").ap(),
    }
    with tile.TileContext(nc) as tc:
        if reps == 1:
            with ExitStack() as ctx:
                emit(ctx, tc, io)
        else:
            with tc.For_i(0, reps, 1):
                with ExitStack() as ctx:
                    emit(ctx, tc, io)
    nc.compile()
    return nc


def host_inputs(query, key, value, Wq, bq, Wk, bk, Wv, bv, Wm, bm):
    """Host-side prep: head-deinterleaving permutation + scale/bias folding +
    bf16 conversion + weight packing.

    Returns (shared weight map, list of per-core input maps)."""
    f = np.float32
    t = np.arange(D)
    perm = (t % DH) * H + t // DH  # row t = head-major channel -> original dm

    Wq = np.asarray(Wq, f); Wk = np.asarray(Wk, f); Wv = np.asarray(Wv, f)
    Wm = np.asarray(Wm, f)
    bq = np.asarray(bq, f); bk = np.asarray(bk, f); bv = np.asarray(bv, f)
    bm = np.asarray(bm, f)

    scale = f(1.0 / np.sqrt(DH))
    wqT = Wq.T[:, perm] * scale      # [i, o'] head-major columns
    wkT = Wk.T[:, perm]
    wvT = Wv.T[:, perm]
    wmT = Wm.T[perm, :]              # [c' head-major, o]
    wpack = np.empty((PC, 8 * D), NPBF16)
    for kc in range(KC):
        wpack[:, (0 + kc) * D:(1 + kc) * D] = wqT[kc * PC:(kc + 1) * PC, :]
        wpack[:, (2 + kc) * D:(3 + kc) * D] = wkT[kc * PC:(kc + 1) * PC, :]
        wpack[:, (4 + kc) * D:(5 + kc) * D] = wvT[kc * PC:(kc + 1) * PC, :]
        wpack[:, (6 + kc) * D:(7 + kc) * D] = wmT[kc * PC:(kc + 1) * PC, :]

    bq_eff = bq[perm] * scale
    bk_eff = bk[perm]
    bm_eff = bm + Wm @ bv
    bpack = np.empty((PC, 7), f)
    for oc in range(KC):
        bpack[:, 0 + oc] = bq_eff[oc * PC:(oc + 1) * PC]
        bpack[:, 2 + oc] = bk_eff[oc * PC:(oc + 1) * PC]
        bpack[:, 4 + oc] = bm_eff[oc * PC:(oc + 1) * PC]
    bpack[:, 6] = 1.0

    shared = {"wpack": wpack, "bpack": bpack}
    query = np.asarray(query, f); key = np.asarray(key, f)
    value = np.asarray(value, f)
    in_maps = []
    for b in range(B):
        m = dict(shared)
        m["xq"] = np.ascontiguousarray(query[b]).astype(NPBF16)
        m["xk"] = np.ascontiguousarray(key[b]).astype(NPBF16)
        m["xv"] = np.ascontiguousarray(value[b]).astype(NPBF16)
        in_maps.append(m)
    return shared, in_maps


_NC = None


def get_nc():
    global _NC
    if _NC is None:
        _NC = build_nc()
    return _NC


def kernel(query, key, value, Wq, bq, Wk, bk, Wv, bv, Wm, bm):
    nc = get_nc()
    _, in_maps = host_inputs(query, key, value, Wq, bq, Wk, bk, Wv, bv, Wm, bm)
    res = run_bass_kernel_spmd(nc, in_maps, core_ids=list(range(B)))
    return np.stack([res.results[b]["out"] for b in range(B)], axis=0)
